# revision 22
# baseline (speedup 1.0000x reference)
"""Multi-head attention (B=16, N=1024, E=768, H=12) on 8 TRN2 NeuronCores.

Data parallel over batch (2 per core, no collectives). Per-core fused kernel:
  - X^T built with 96 PE transposes (bf16), drained by the scalar engine.
  - QKV: Q/K feature-major (each 128-chunk = two heads' Q^T/K^T, bias added
    on the scalar engine from PSUM); V token-major into a (tok, 12*65)
    layout with a constant ones column per head.
  - energy^T per head pair as two concurrent row-tiled matmuls (K=64 at row
    offsets 0/64) into one (128,1024) PSUM tile; a single Exp ACTIVATE
    (scale=1/8, no max subtraction - |logit| < 2 by construction) drains it.
  - attn@V: lhsT = [V | 1] (M=65); PSUM row 64 = softmax denominators.
  - Normalization: reciprocal_approx_fast + gpsimd partition broadcast,
    fused into the PSUM->SBUF copy that writes the shifted-duplicate
    "DOUBLE" layout; stride-12 APs over DOUBLE give exact 128-row slabs of
    Y^T for the reference's scrambled (H,N,D)->(N,E) reshape, so the out
    projection is 6 clean K=128 accumulating matmuls per 128-token tile.
  - Software pipelining: attn@V matmuls of sub-group g-1 are interleaved
    into the energy matmuls of sub-group g (and batch-0 out-projection into
    batch-1 attention) so the PE never stalls on the exp drain.
"""

import contextlib

import numpy as np

import concourse.bass as bass
import concourse.tile as tile
import concourse.mybir as mybir
from concourse import bacc
from concourse import bass_utils

B, N, E, H = 16, 1024, 768, 12
D = E // H          # 64
N_CORES = 8
BPC = B // N_CORES  # 2
T = BPC * N         # 2048
F3 = 3 * E
SCALE = 1.0 / float(np.sqrt(np.float32(D)))

FP32 = mybir.dt.float32
BF16 = mybir.dt.bfloat16
INT32 = mybir.dt.int32
AF = mybir.ActivationFunctionType
OP = mybir.AluOpType


def _emit(tc, x_ap, wqkv_ap, bqkv_ap, wout_ap, bout_ap, out_ap):
    nc = tc.nc
    EC = E // 128      # 6
    FC = 2 * E // 128  # 12
    TC4 = T // 512     # 4
    TC16 = T // 128    # 16
    HM = H * N         # 12288

    stack = contextlib.ExitStack()
    with stack:
        const_pool = stack.enter_context(tc.tile_pool(name="const", bufs=1))
        w_pool = stack.enter_context(tc.tile_pool(name="w", bufs=1))
        qkt_pool = stack.enter_context(tc.tile_pool(name="qkt", bufs=1))
        vo_pool = stack.enter_context(tc.tile_pool(name="vo", bufs=1))
        dbl_pool = stack.enter_context(tc.tile_pool(name="dbl", bufs=1))

        pse = stack.enter_context(
            tc.tile_pool(name="pse", bufs=2, space="PSUM"))   # (128,1024) = 2 banks
        pso = stack.enter_context(
            tc.tile_pool(name="pso", bufs=4, space="PSUM"))   # (65,512) = 1 bank

        # ---- constants --------------------------------------------------
        bq = const_pool.tile([128, FC], FP32, tag="bq")
        nc.sync.dma_start(bq[:, :], bqkv_ap.rearrange("(c p) -> p c", p=128)[:, 0:FC])
        bv_row = const_pool.tile([1, E], FP32, tag="brow", name="bv_row")
        nc.sync.dma_start(bv_row[:, :], bqkv_ap[2 * E:3 * E].unsqueeze(0))
        bv = const_pool.tile([128, E], FP32, tag="bv")
        nc.gpsimd.partition_broadcast(bv[:, :], bv_row[:, :], channels=128)
        bo_row = const_pool.tile([1, E], FP32, tag="brow", name="bo_row")
        nc.sync.dma_start(bo_row[:, :], bout_ap.unsqueeze(0))
        bo = const_pool.tile([128, E], FP32, tag="bo")
        nc.gpsimd.partition_broadcast(bo[:, :], bo_row[:, :], channels=128)
        # identity (bf16) for PE transposes
        iota_f = const_pool.tile([128, 128], FP32, tag="iota_f")
        nc.gpsimd.iota(iota_f[:, :], pattern=[[1, 128]], channel_multiplier=0,
                       allow_small_or_imprecise_dtypes=True)
        iota_p = const_pool.tile([128, 1], FP32, tag="iota_p")
        nc.gpsimd.iota(iota_p[:, :], pattern=[[0, 1]], channel_multiplier=1,
                       allow_small_or_imprecise_dtypes=True)
        idt = const_pool.tile([128, 128], FP32, tag="idt")
        nc.vector.tensor_scalar(idt[:, :], iota_f[:, :], iota_p[:, :], None,
                                op0=OP.is_equal)

        # ---- weights: V columns first so QKV-V can start early ----------
        wsb = [w_pool.tile([128, F3], BF16, tag=f"wsb{ec}", name=f"wsb{ec}")
               for ec in range(EC)]
        wosb = [w_pool.tile([128, E], BF16, tag=f"wosb{ec}", name=f"wosb{ec}")
                for ec in range(EC)]
        with tc.tile_pool(name="wstage", bufs=2) as wstage:
            for ec in range(EC):
                wfv = wstage.tile([128, E], FP32, tag="wfv")
                nc.scalar.dma_start(wfv[:, :], wqkv_ap[ec * 128:(ec + 1) * 128,
                                                       2 * E:3 * E])
                nc.vector.tensor_copy(wsb[ec][:, 2 * E:3 * E], wfv[:, :])
            for ec in range(EC):
                wfq = wstage.tile([128, 2 * E], FP32, tag="wfq")
                nc.scalar.dma_start(wfq[:, :], wqkv_ap[ec * 128:(ec + 1) * 128,
                                                       0:2 * E])
                nc.vector.tensor_copy(wsb[ec][:, 0:2 * E], wfq[:, :])
            for ec in range(EC):
                wf2 = wstage.tile([128, E], FP32, tag="wfv")
                nc.scalar.dma_start(wf2[:, :], wout_ap[ec * 128:(ec + 1) * 128, :])
                nc.vector.tensor_copy(wosb[ec][:, :], wf2[:, :])

        # ---- X load + cast + PE transpose -> Xt -------------------------
        xt_pool = stack.enter_context(tc.tile_pool(name="xt", bufs=1))
        with contextlib.nullcontext():
            xt = [xt_pool.tile([128, T], BF16, tag=f"xt{ec}", name=f"xt{ec}")
                  for ec in range(EC)]
            with tc.tile_pool(name="xstage", bufs=3) as xstage:
                for tc16 in range(TC16):
                    xf = xstage.tile([128, E], FP32, tag="xf")
                    xeng = (nc.sync, nc.gpsimd)[tc16 % 2]
                    xeng.dma_start(
                        xf[:, :], x_ap[tc16 * 128:(tc16 + 1) * 128, :])
                    pt = pse.tile([128, 1024], FP32, tag="pse")
                    for ec in range(EC):
                        nc.tensor.transpose(
                            pt[:, ec * 128:(ec + 1) * 128],
                            xf[:, ec * 128:(ec + 1) * 128], idt[:, :])
                    for ec in range(EC):
                        nc.scalar.copy(
                            xt[ec][:, tc16 * 128:(tc16 + 1) * 128],
                            pt[:, ec * 128:(ec + 1) * 128])

            # ---- QKV: V path -> VO (tok-major, ones col per head) -------
            vo = [vo_pool.tile([128, H * (D + 1)], BF16, tag=f"vo{i}",
                               name=f"vo{i}") for i in range(TC16)]
            for tc16 in range(TC16):
                ps = pse.tile([128, 1024], FP32, tag="pse")
                for ec in range(EC):
                    nc.tensor.matmul(
                        ps[:, 0:512],
                        xt[ec][:, tc16 * 128:(tc16 + 1) * 128],
                        wsb[ec][:, 2 * E:2 * E + 512],
                        start=(ec == 0), stop=(ec == EC - 1))
                for ec in range(EC):
                    nc.tensor.matmul(
                        ps[:, 512:768],
                        xt[ec][:, tc16 * 128:(tc16 + 1) * 128],
                        wsb[ec][:, 2 * E + 512:3 * E],
                        start=(ec == 0), stop=(ec == EC - 1))
                nc.vector.memset(vo[tc16][:, D::(D + 1)], 1.0)
                vo3a = vo[tc16][:, 0:8 * (D + 1)].rearrange(
                    "p (h j) -> p h j", j=D + 1)[:, :, 0:D]
                nc.vector.tensor_tensor(
                    vo3a, ps[:, 0:512].rearrange("p (h j) -> p h j", j=D),
                    bv[:, 0:512].rearrange("p (h j) -> p h j", j=D), op=OP.add)
                vo3b = vo[tc16][:, 8 * (D + 1):].rearrange(
                    "p (h j) -> p h j", j=D + 1)[:, :, 0:D]
                nc.vector.tensor_tensor(
                    vo3b, ps[:, 512:768].rearrange("p (h j) -> p h j", j=D),
                    bv[:, 512:768].rearrange("p (h j) -> p h j", j=D), op=OP.add)

            # ---- QKV: Q/K path (feature-major), bias via ACT ------------
            qkt = [qkt_pool.tile([128, T], BF16, tag=f"qkt{fc}",
                                 name=f"qkt{fc}") for fc in range(FC)]
            for fc in range(FC):
                for tch in range(TC4):
                    ps = pse.tile([128, 1024], FP32, tag="pse")
                    for ec in range(EC):
                        nc.tensor.matmul(
                            ps[:, 0:512],
                            wsb[ec][:, fc * 128:(fc + 1) * 128],
                            xt[ec][:, tch * 512:(tch + 1) * 512],
                            start=(ec == 0), stop=(ec == EC - 1))
                    nc.vector.tensor_scalar_add(
                        qkt[fc][:, tch * 512:(tch + 1) * 512], ps[:, 0:512],
                        bq[:, fc:fc + 1])

        # ---- attention + out projection, software pipelined -------------
        et_pool = stack.enter_context(tc.tile_pool(name="et", bufs=13))
        small_pool = stack.enter_context(tc.tile_pool(name="small", bufs=1))
        rb_pool = stack.enter_context(tc.tile_pool(name="rb", bufs=2))
        osb_pool = stack.enter_context(tc.tile_pool(name="osb", bufs=2))

        dbl = [dbl_pool.tile([128, HM], BF16, tag=f"dbl{b}", name=f"dbl{b}")
               for b in range(BPC)]

        def alloc_pos():
            return [pso.tile([65, 512], FP32, tag="po", name=f"po{h}")
                    for h in range(2)]

        def emit_attnv_tk(st, pos, tk):
            """attn@V matmuls (both halves) for one tk chunk of sub-group st."""
            b, fc, tq, ets = st
            for half in range(2):
                h = 2 * fc + half
                nc.tensor.matmul(
                    pos[half][:, :],
                    vo[b * 8 + tk][:, h * (D + 1):(h + 1) * (D + 1)],
                    ets[tk][:, half * 512:(half + 1) * 512],
                    start=(tk == 0), stop=(tk == 7))

        def emit_drain(st, pos):
            b, fc, tq, _ = st
            for half in range(2):
                h = 2 * fc + half
                po = pos[half]
                sraw = small_pool.tile([1, 512], FP32, tag="sraw")
                nc.vector.tensor_copy(sraw[:, :], po[D:D + 1, :])
                rec = small_pool.tile([1, 512], FP32, tag="rec")
                nc.vector.reciprocal_approx_fast(rec[:, :], sraw[:, :])
                rb = rb_pool.tile([64, 512], FP32, tag="rb")
                nc.gpsimd.partition_broadcast(rb[:, :], rec[:, :], channels=64)
                m0 = h * N + tq * 512
                nc.vector.tensor_tensor(
                    dbl[b][0:D, m0:m0 + 512], po[0:D, :], rb[:, :], op=OP.mult)
                if m0 == 0:
                    nc.vector.tensor_tensor(
                        dbl[b][D:128, 0:511], po[0:D, 1:512], rb[:, 1:512],
                        op=OP.mult)
                else:
                    nc.vector.tensor_tensor(
                        dbl[b][D:128, m0 - 1:m0 + 511], po[0:D, :], rb[:, :],
                        op=OP.mult)

        def emit_outproj_chunk(b, npc):
            pf = pse.tile([128, 1024], FP32, tag="pse")
            for cc in range(EC):
                off = 2 * cc + 12 * (npc * 128)
                lhsT = dbl[b][:, off::12][:, 0:128]
                nc.tensor.matmul(pf[:, 0:512], lhsT, wosb[cc][:, 0:512],
                                 start=(cc == 0), stop=(cc == EC - 1))
            for cc in range(EC):
                off = 2 * cc + 12 * (npc * 128)
                lhsT = dbl[b][:, off::12][:, 0:128]
                nc.tensor.matmul(pf[:, 512:768], lhsT, wosb[cc][:, 512:768],
                                 start=(cc == 0), stop=(cc == EC - 1))
            osb = osb_pool.tile([128, E], FP32, tag="osb")
            nc.vector.tensor_tensor(osb[:, :], pf[:, 0:768], bo[:, :], op=OP.add)
            nc.sync.dma_start(
                out_ap[b * N + npc * 128:b * N + (npc + 1) * 128, :], osb[:, :])

        prev = None           # (b, fc, tq, ets) awaiting attn@V
        op_queue = []         # deferred (b, npc) out-projection chunks
        subgroup_i = 0
        for b in range(BPC):
            for fc in range(H // 2):
                for tq in range(2):
                    ets = []
                    prev_pos = alloc_pos() if prev is not None else None
                    for tk in range(8):
                        pe = pse.tile([128, 1024], FP32, tag="pse")
                        for half in range(2):
                            lo = 64 * half
                            nc.tensor.matmul(
                                pe[:, half * 512:(half + 1) * 512],
                                qkt[6 + fc][lo:lo + 64,
                                            b * N + tk * 128:b * N + (tk + 1) * 128],
                                qkt[fc][lo:lo + 64,
                                        b * N + tq * 512:b * N + (tq + 1) * 512],
                                start=True, stop=True)
                        et = et_pool.tile([128, 1024], BF16, tag="et")
                        nc.scalar.activation(et[:, :], pe[:, :], AF.Exp,
                                             bias=0.0, scale=SCALE)
                        ets.append(et)
                        if prev is not None:
                            emit_attnv_tk(prev, prev_pos, tk)
                    if prev is not None:
                        emit_drain(prev, prev_pos)
                        prev = None
                    # interleave batch-0 out-projection into batch-1 attention
                    if b == 1 and subgroup_i >= 13 and op_queue:
                        emit_outproj_chunk(*op_queue.pop(0))
                        if subgroup_i >= 20 and op_queue:
                            emit_outproj_chunk(*op_queue.pop(0))
                    prev = (b, fc, tq, ets)
                    subgroup_i += 1
            for npc in range(N // 128):
                op_queue.append((b, npc))
        prev_pos = alloc_pos()
        for tk in range(8):
            emit_attnv_tk(prev, prev_pos, tk)
        emit_drain(prev, prev_pos)
        while op_queue:
            emit_outproj_chunk(*op_queue.pop(0))


_built = None


def _build():
    global _built
    if _built is not None:
        return _built
    nc = bacc.Bacc("TRN2", target_bir_lowering=False, debug=False,
                   num_devices=N_CORES)
    x_ap = nc.dram_tensor("x", (T, E), FP32, kind="ExternalInput").ap()
    wqkv_ap = nc.dram_tensor("w_qkv", (E, F3), FP32, kind="ExternalInput").ap()
    bqkv_ap = nc.dram_tensor("b_qkv", (F3,), FP32, kind="ExternalInput").ap()
    wout_ap = nc.dram_tensor("w_out", (E, E), FP32, kind="ExternalInput").ap()
    bout_ap = nc.dram_tensor("b_out", (E,), FP32, kind="ExternalInput").ap()
    out_ap = nc.dram_tensor("out", (T, E), FP32, kind="ExternalOutput").ap()
    with tile.TileContext(nc) as tc:
        _emit(tc, x_ap, wqkv_ap, bqkv_ap, wout_ap, bout_ap, out_ap)
    nc.compile()
    _built = nc
    return nc


def kernel(x, W_qkv, b_qkv, W_out, b_out, _trace=False):
    x = np.ascontiguousarray(np.asarray(x, dtype=np.float32))
    W_qkv = np.ascontiguousarray(np.asarray(W_qkv, dtype=np.float32))
    b_qkv = np.ascontiguousarray(np.asarray(b_qkv, dtype=np.float32))
    W_out = np.ascontiguousarray(np.asarray(W_out, dtype=np.float32))
    b_out = np.ascontiguousarray(np.asarray(b_out, dtype=np.float32))

    nc = _build()
    in_maps = [
        {
            "x": x[c * BPC:(c + 1) * BPC].reshape(T, E),
            "w_qkv": W_qkv, "b_qkv": b_qkv, "w_out": W_out, "b_out": b_out,
        }
        for c in range(N_CORES)
    ]
    res = bass_utils.run_bass_kernel_spmd(
        nc, in_maps, core_ids=list(range(N_CORES)), trace=_trace)
    out = np.concatenate(
        [res.results[c]["out"].reshape(BPC, N, E) for c in range(N_CORES)],
        axis=0)
    if _trace:
        kernel._last_results = res
    return out
